# revision 23
# baseline (speedup 1.0000x reference)
"""Multi-head attention (B=4, S=2048, D=1024, H=16) on 8 trn2 NeuronCores.

Sharding: core c handles batch b=c//2 and head-group g=c%2 (8 of 16 heads).
Each core computes its head-group's Q/K/V projections, attention, and the
partial output projection (its 512 columns of Wo). The host sums the two
partial outputs per batch and adds bo.

v3: single software-pipelined emission stream sized to keep the scalar
engine (exp: 256 calls x ~1.11us = the true bottleneck) saturated:
  - "triangle" schedule: window w runs every attention chunk whose newest
    dependency is token-block w (new k-chunks of old q-blocks + the catch-up
    q-block w); projections for block w+1 (next rep's block 0 at w=3) drain
    as filler between kt-steps so the PE never idles >3.4us (HAM stays warm).
  - cross-rep rebalance: window 3's ACT surplus is shaved by deferring the
    chunks that read only blocks >= 2 — (2,pr) kts 12-16 and (3,pr>=2) kts
    8-16 — into the next rep's ACT-starved window 0.
  - within each chunk the score pair + exp of kt are emitted before the PVs
    of kt-1 so the two score matmuls stay adjacent in the in-order PE stream
    (they run concurrently in PE row groups 0/64).
  - PV accumulates into one PSUM slot [P,2,512] (both heads, col 0-64 rows);
    partial chunk sums are evicted to per-unit SBUF accumulators (bf16) so 2
    PSUM banks serve all 16 in-flight (qb, pair) units.
  - tails (softmax normalize via ones-column row 64) and out-projections are
    filler generators drained between kt-steps, off the exp critical path,
    with spacer yields so their PE ops never head-of-line block the PE queue
    behind a congested DVE.

v3.1: z/zt/ofin partials in bf16 (halves output DMA + DVE copy cost;
rel_err 4.7e-3 -> 5.0e-3, gate is 2e-2).

v3.2: PV_LAG=3 — each PV trails its exp by 3 kt-steps in the PE stream
(at lag 1 the PE reaches PV(kt-1) ~100-300ns before its ~1.1us exp has
retired and stalls on the ACT sem; ~5us/rep, A/B-verified lag 3 > 2 > 1 > 4
on interleaved r=9 totals). drain(3) per kt-step re-confirmed optimal
(2 and 4 are each ~8us/rep worse).

Measured sustained engine rates (delta-N at r=9, cancels launch overhead;
burst/boost-clock readings from short r<=5 chains are ~2x optimistic and
NOT representative):
  - ACT exp [128,1024] f32-PSUM -> bf16-SBUF: ~1.1-1.2us/call; bf16-SBUF
    input is only ~10% faster sustained (the apparent 2x was boost-clock),
    so DVE-staging scores to bf16 (~1.25us/copy) is a strict loss.
  - PE bf16 matmul ~(moving_rows + ldw_cols) * 0.417ns sustained; per-rep
    PE busy ~330-350us (proj 102, scores 68 (row-group-packed pairs), PV
    123 (m=65 of 128, unpackable: one moving stream per PE pass), outproj
    34) => PE-bound at ~93% occupancy (middle-rep timeline sim), ACT ~75%.
  - fp8/DoubleRow anywhere in the attention chain injects ~3.5-5% rel err
    (multiplicative weight noise does not average out in random-sign
    weighted sums) vs the 2e-2 gate -> unusable. DVE tensor_scalar pow is
    rejected by the ISA (no 2^x path off the ACT engine); Pool/gpsimd
    tensor ops cannot read PSUM or cast dtypes.
Remaining theoretical headroom is <8% (PE roofline) absent a PSUM-larger
or fp8-tolerant reformulation. Also tried and reverted: replacing the 32
bn broadcast matmuls (~7us PE) with gpsimd partition_broadcast — needs a
p64->p0 sbuf DMA hop first (the op only reads partition 0, and DVE ops
cannot shift base partitions), and the longer DMA->Pool->DVE tail chain
measured ~17us/rep WORSE than the K=1 matmul (A/B r9 totals).
"""

import collections
import numpy as np
import ml_dtypes

B, S, D, H, HD = 4, 2048, 1024, 16, 64
NCORES = 8
FG = 512      # head-group width per core (8 heads x 64)
NPAIR = 4     # head pairs per core
KT_D = 8      # D / 128 contraction tiles
MT = 4        # FG / 128 output tiles
TB = 4        # token blocks of 512
NT = 16       # token tiles of 128
P = 128
PV_LAG = 3   # kt-steps each PV trails its exp in the PE stream
DRN = 3      # filler micro-steps drained per kt-step

_CACHE: dict = {}


def _build_nc(repeat=1):
    import concourse.mybir as mybir
    import concourse.tile as tile
    from concourse import bacc

    dt = mybir.dt
    BF = dt.bfloat16
    F32 = dt.float32
    Exp = mybir.ActivationFunctionType.Exp

    nc = bacc.Bacc(None, target_bir_lowering=False)

    xqT = nc.dram_tensor("xqT", [D, S], BF, kind="ExternalInput")
    xkT = nc.dram_tensor("xkT", [D, S], BF, kind="ExternalInput")
    xvT = nc.dram_tensor("xvT", [D, S], BF, kind="ExternalInput")
    wqT = nc.dram_tensor("wqT", [D, FG], BF, kind="ExternalInput")
    wkT = nc.dram_tensor("wkT", [D, FG], BF, kind="ExternalInput")
    wvT = nc.dram_tensor("wvT", [D, FG], BF, kind="ExternalInput")
    woT = nc.dram_tensor("woT", [FG, D], BF, kind="ExternalInput")
    bq2 = nc.dram_tensor("bq2", [P, MT], F32, kind="ExternalInput")
    bk2 = nc.dram_tensor("bk2", [P, MT], F32, kind="ExternalInput")
    bvr = nc.dram_tensor("bvr", [1, FG], BF, kind="ExternalInput")
    z = nc.dram_tensor("z", [S, D], BF, kind="ExternalOutput")

    xqT_v = xqT.rearrange("(ko p) s -> p ko s", p=P)
    xkT_v = xkT.rearrange("(ko p) s -> p ko s", p=P)
    xvT_v = xvT.rearrange("(ko p) s -> p ko s", p=P)
    wqT_v = wqT.rearrange("(ko p) m -> p ko m", p=P)
    wkT_v = wkT.rearrange("(ko p) m -> p ko m", p=P)
    wvT_v = wvT.rearrange("(ko p) m -> p ko m", p=P)
    woT_v = woT.rearrange("(ko p) n -> p ko n", p=P)
    z_v = z.rearrange("(t p) n -> t p n", p=P)

    with tile.TileContext(nc) as tc:
        with (
            tc.tile_pool(name="const", bufs=1) as constp,
            tc.tile_pool(name="xc", bufs=2) as xpool,
            tc.tile_pool(name="big", bufs=1) as bigp,
            tc.tile_pool(name="pt", bufs=6) as ppool,
            tc.tile_pool(name="sm", bufs=2) as smallp,
            tc.tile_pool(name="zs", bufs=2) as zpool,
            tc.tile_pool(name="ps", bufs=2, space="PSUM") as psum,
        ):
            wk_sb = constp.tile([P, KT_D, FG], BF)
            nc.sync.dma_start(wk_sb[:], wkT_v[:])
            wv_sb = constp.tile([P, KT_D, FG], BF)
            nc.sync.dma_start(wv_sb[:], wvT_v[:])
            wq_sb = constp.tile([P, KT_D, FG], BF)
            nc.sync.dma_start(wq_sb[:], wqT_v[:])
            bqs = constp.tile([P, MT], F32)
            nc.sync.dma_start(bqs[:], bq2[:])
            bks = constp.tile([P, MT], F32)
            nc.sync.dma_start(bks[:], bk2[:])
            bvrow = constp.tile([1, FG], BF)
            nc.sync.dma_start(bvrow[:], bvr[:])
            wo_sb = constp.tile([P, MT, D], BF)
            nc.sync.dma_start(wo_sb[:], woT_v[:])
            ones_c = constp.tile([1, P], BF)
            nc.vector.memset(ones_c[:], 1.0)
            # ones rows AT partition 64, matching the normalizer row of the
            # o accumulators (matmul operands must be partition-aligned)
            ones65c = constp.tile([HD + 1, P], BF)
            nc.vector.memset(ones65c[:], 1.0)
            ones65f = constp.tile([HD + 1, P], F32)
            nc.vector.memset(ones65f[:], 1.0)

            # broadcast bv across all 128 partitions once: [128, 512] f32
            bv_bc = constp.tile([P, FG], F32)
            bv_ps = psum.tile([P, 512], F32, tag="proj")
            nc.tensor.matmul(bv_ps[:], ones_c[:], bvrow[:], start=True, stop=True)
            nc.vector.tensor_copy(bv_bc[:], bv_ps[:])

            qT = bigp.tile([P, MT, S], BF)
            kT = bigp.tile([P, MT, S], BF)
            v_sb = bigp.tile([P, NT, 2 * NPAIR, HD + 1], BF)
            y_sb = bigp.tile([P, MT, S], BF)
            nc.vector.memset(v_sb[:, :, :, HD:HD + 1], 1.0)

            # per-unit partial O accumulators (only q-blocks 0..2 span
            # multiple windows); rows 0-63 = V part (per head), row 64 = the
            # softmax normalizer. bf16 keeps SBUF inside budget.
            osb = {}
            for qb in range(TB):
                for pr in range(NPAIR):
                    if qb == TB - 1 and pr < 2:
                        continue  # (3, 0/1) always run single-chunk
                    t_ = constp.tile([HD + 1, 2, 512], BF, name=f"osb{qb}{pr}")
                    osb[(qb, pr)] = t_

            # ---------------- filler machinery ----------------
            # entries: [seq, gen] where seq is a global block sequence number
            # (rep*TB + block) for projection generators, None for tails /
            # out-projections. Emission order defines the dependency graph,
            # so force_block() must fully emit the projections of any block
            # before a chunk that reads it is emitted; drain() merely paces
            # the default (priority-setting) interleave.
            fill = collections.deque()

            def drain(n):
                while n > 0 and fill:
                    try:
                        next(fill[0][1])
                        n -= 1
                    except StopIteration:
                        fill.popleft()

            def force_block(seq):
                while any(e[0] is not None and e[0] <= seq for e in fill):
                    try:
                        next(fill[0][1])
                    except StopIteration:
                        fill.popleft()

            def force_entry(e):
                # FIFO: drain from the front until entry e has been consumed
                while e is not None and e in fill:
                    try:
                        next(fill[0][1])
                    except StopIteration:
                        fill.popleft()

            def drain_all():
                while fill:
                    try:
                        next(fill[0][1])
                    except StopIteration:
                        fill.popleft()

            # ---------------- projections (filler generators) ----------------
            def gen_kqproj(w_sb, bias, dstT, tb, x_t, m):
                ps = psum.tile([P, 512], F32, tag="proj", name="pps")
                for kt in range(KT_D):
                    nc.tensor.matmul(
                        ps[:], w_sb[:, kt, m * 128:(m + 1) * 128],
                        x_t[:, kt, :],
                        start=(kt == 0), stop=(kt == KT_D - 1))
                    yield
                nc.vector.tensor_scalar_add(
                    dstT[:, m, tb * 512:(tb + 1) * 512], ps[:], bias[:, m:m + 1])
                yield

            def gen_vproj(tb, x_t, tt):
                ps = psum.tile([P, 512], F32, tag="proj", name="vps")
                for kt in range(KT_D):
                    nc.tensor.matmul(
                        ps[:], x_t[:, kt, tt * 128:(tt + 1) * 128],
                        wv_sb[:, kt, :],
                        start=(kt == 0), stop=(kt == KT_D - 1))
                    yield
                nc.vector.tensor_add(
                    v_sb[:, tb * 4 + tt, :, 0:HD],
                    ps[:].rearrange("p (h d) -> p h d", h=2 * NPAIR),
                    bv_bc[:].rearrange("p (h d) -> p h d", h=2 * NPAIR))
                yield

            def push_block_proj(seq, tb):
                xk_t = xpool.tile([P, KT_D, 512], BF, tag="xk", name="xk_t")
                xv_t = xpool.tile([P, KT_D, 512], BF, tag="xv", name="xv_t")
                xq_t = xpool.tile([P, KT_D, 512], BF, tag="xq", name="xq_t")
                for xt, xv_ in ((xk_t, xkT_v), (xv_t, xvT_v), (xq_t, xqT_v)):
                    half = KT_D // 2
                    nc.sync.dma_start(
                        xt[:, 0:half, :], xv_[:, 0:half, tb * 512:(tb + 1) * 512])
                    nc.sync.dma_start(
                        xt[:, half:, :], xv_[:, half:, tb * 512:(tb + 1) * 512])
                for m in range(MT):
                    fill.append([seq, gen_kqproj(wk_sb, bks, kT, tb, xk_t, m)])
                for tt in range(4):
                    fill.append([seq, gen_vproj(tb, xv_t, tt)])
                for m in range(MT):
                    fill.append([seq, gen_kqproj(wq_sb, bqs, qT, tb, xq_t, m)])

            # ---------------- out-projection (filler) ----------------
            def gen_out_proj(t):
                # spacer: let the y_sb writes (DVE muls) clear before these
                # PE matmuls enter the in-order PE stream
                for _ in range(4):
                    yield
                zt = zpool.tile([P, 2, 512], BF, tag="z", name="zt")
                for nb in range(2):
                    ps = psum.tile([P, 512], F32, tag="proj", name="zps")
                    for kt in range(MT):
                        nc.tensor.matmul(
                            ps[:], y_sb[:, kt, t * 128:(t + 1) * 128],
                            wo_sb[:, kt, nb * 512:(nb + 1) * 512],
                            start=(kt == 0), stop=(kt == MT - 1))
                        yield
                    with nc.allow_low_precision(reason="bf16 z partials"):
                        nc.vector.tensor_copy(zt[:, nb, :], ps[:])
                    yield
                nc.sync.dma_start(z_v[t], zt[:].rearrange("p a b -> p (a b)"))
                yield

            tails_done = {}
            last_tail = {}
            out_entries = {}

            def gen_tail(rep, qb, pr, o_fin):
                # o_fin [65, 2, 512] SBUF: rows 0-63 V-part, row 64 = sum(p).
                # Broadcast the RAW normalizer row to 64 partitions via a K=1
                # matmul (no DVE dependency -> no PE head-of-line block), then
                # reciprocal on 64 active lanes (a [1,N] DVE reciprocal runs
                # on one lane and costs ~6.5us; this costs ~0.7us).
                yield
                yield
                ones = ones65f if o_fin.dtype == F32 else ones65c
                for h01 in range(2):
                    bn = psum.tile([HD, 512], F32, tag="proj", name="bn")
                    nc.tensor.matmul(bn[:], ones[HD:HD + 1, 0:HD],
                                     o_fin[HD:HD + 1, h01, :],
                                     start=True, stop=True)
                    yield
                    rec = psum.tile([HD, 512], F32, tag="proj", name="rec")
                    nc.vector.reciprocal_approx_fast(rec[:], bn[:])
                    yield
                    part = h01 * 64
                    nc.vector.tensor_mul(
                        y_sb[part:part + 64, pr, qb * 512:(qb + 1) * 512],
                        o_fin[0:HD, h01, :], rec[:])
                    yield
                key = (rep, qb)
                n = tails_done[key] = tails_done.get(key, 0) + 1
                if n == NPAIR:
                    ents = []
                    for t in range(qb * 4, qb * 4 + 4):
                        e = [None, gen_out_proj(t)]
                        ents.append(e)
                        fill.append(e)
                    out_entries[key] = ents

            # ---------------- attention chunk ----------------
            def emit_chunk(rep, qb, pr, kts, final):
                # the chunk reads kT/v of blocks up to kts[-1]//4 and qT of
                # block qb: their projections must already be emitted
                force_block(rep * TB + max(qb, kts[-1] // 4))
                if kts[0] == 0:
                    # about to overwrite osb/start this unit's new pass: the
                    # previous rep's tail (which reads osb) must be emitted
                    # first so its read binds to the old data
                    force_entry(last_tail.pop((qb, pr), None))
                o_ps = psum.tile([P, 2, 512], F32, tag="oacc", name="oacc",
                                 bufs=1)

                def pv(i, p_t, kt):
                    for h01 in range(2):
                        nc.tensor.matmul(
                            o_ps[0:HD + 1, h01, :],
                            v_sb[:, kt, 2 * pr + h01, :],
                            p_t[:, h01, :],
                            start=(i == 0), stop=(i == len(kts) - 1))

                # software-pipelined: the score pair + exp of kt are emitted
                # before the PVs of older kts, keeping the score MMs adjacent
                # in the PE stream (they run concurrently in row groups 0/64).
                # PVs lag by PV_LAG kt-steps so the in-order PE stream never
                # reaches a PV before its ~1.1us exp has retired (at lag 1 the
                # PE arrives ~100-300ns early and stalls on the ACT sem).
                pend = collections.deque()
                for i, kt in enumerate(kts):
                    s_ps = psum.tile([P, 2, 512], F32, tag="scores", name="sps")
                    for h01 in range(2):
                        nc.tensor.matmul(
                            s_ps[:, h01, :],
                            kT[h01 * 64:(h01 + 1) * 64, pr,
                               kt * 128:(kt + 1) * 128],
                            qT[h01 * 64:(h01 + 1) * 64, pr,
                               qb * 512:(qb + 1) * 512],
                            start=True, stop=True)
                    p_t = ppool.tile([P, 2, 512], BF, tag="pt", name="p_t")
                    nc.scalar.activation(p_t[:], s_ps[:], Exp, scale=0.125)
                    pend.append((i, p_t, kt))
                    if len(pend) > PV_LAG:
                        pv(*pend.popleft())
                    drain(DRN)
                while pend:
                    pv(*pend.popleft())
                # evict o_ps (frees the single oacc slot for the next chunk)
                first = kts[0] == 0
                if final:
                    # previous rep's out-proj (reads y_sb[:, :, qb block])
                    # must be emitted before this rep's tail overwrites it
                    for e in out_entries.pop((rep - 1, qb), []):
                        force_entry(e)
                if final and first:
                    # qb == 3 units run all 16 kts in one chunk: evict to a
                    # private staging tile and queue the tail on it.
                    o_fin = smallp.tile([HD + 1, 2, 512], BF, tag="ofin",
                                        name="ofin")
                    with nc.allow_low_precision(reason="bf16 O staging"):
                        nc.vector.tensor_copy(o_fin[:], o_ps[0:HD + 1, :, :])
                    e = [None, gen_tail(rep, qb, pr, o_fin)]
                    last_tail[(qb, pr)] = e
                    fill.append(e)
                else:
                    # accumulate into the unit's own SBUF tile (no ring
                    # contention: the tail can drain arbitrarily late)
                    dst = osb[(qb, pr)]
                    with nc.allow_low_precision(reason="bf16 partial O acc"):
                        if first:
                            nc.vector.tensor_copy(dst[:], o_ps[0:HD + 1, :, :])
                        else:
                            nc.vector.tensor_add(
                                dst[:], o_ps[0:HD + 1, :, :], dst[:])
                    if final:
                        e = [None, gen_tail(rep, qb, pr, dst)]
                        last_tail[(qb, pr)] = e
                        fill.append(e)

            # ---------------- the pipelined schedule ----------------
            # window 3 is ACT-heavy and window 0 ACT-starved, so part of the
            # (qb=3, pr>=2) catch-up work is deferred into the next rep's
            # window 0 (it reads only blocks >= 1, which the next rep's
            # projections overwrite later).
            deferred = []
            prio_w3 = None
            for rep in range(repeat):
                if rep == 0:
                    push_block_proj(0, 0)
                for w in range(TB):
                    if w == 0:
                        # deferred chunks first: their reads must bind to the
                        # PREVIOUS rep's kT/qT/v before this rep's projection
                        # pushes overwrite blocks 1..3. Priority-boost them to
                        # the previous rep's w3 so the scheduler prefers their
                        # scores over leftover projection fillers at the rep
                        # boundary (else ACT starves ~26us per rep).
                        if deferred and prio_w3 is not None:
                            off = tc.cur_priority - prio_w3 + 500
                            with tc.high_priority(offset=off):
                                for prep, dqb, dpr, dkts in deferred:
                                    emit_chunk(prep, dqb, dpr, dkts,
                                               final=True)
                        else:
                            for prep, dqb, dpr, dkts in deferred:
                                emit_chunk(prep, dqb, dpr, dkts, final=True)
                        deferred = []
                    if w == TB - 1:
                        prio_w3 = tc.cur_priority
                    # queue next block's projections (next rep's block 0 at w=3)
                    if w < TB - 1:
                        push_block_proj(rep * TB + w + 1, w + 1)
                    elif rep + 1 < repeat:
                        push_block_proj((rep + 1) * TB, 0)
                    # new k-chunk for older q-blocks first: they need only
                    # this window's K (drained first from the filler queue)
                    for qb in range(w):
                        for pr in range(NPAIR):
                            if w == TB - 1 and qb == 2 and rep + 1 < repeat:
                                # deferred work may only read blocks >= 2
                                # (the next rep's w0 filler overwrites b1)
                                deferred.append(
                                    (rep, qb, pr, list(range(12, 16))))
                                continue
                            emit_chunk(rep, qb, pr,
                                       list(range(4 * w, 4 * w + 4)),
                                       final=(w == TB - 1))
                    # catch-up unit (qb=w) last: needs this window's Q
                    for pr in range(NPAIR):
                        if w == TB - 1 and pr >= 2 and rep + 1 < repeat:
                            emit_chunk(rep, w, pr, list(range(8)),
                                       final=False)
                            deferred.append(
                                (rep, w, pr, list(range(8, 4 * TB))))
                        else:
                            emit_chunk(rep, w, pr, list(range(4 * (w + 1))),
                                       final=(w == TB - 1))
            for prep, dqb, dpr, dkts in deferred:
                emit_chunk(prep, dqb, dpr, dkts, final=True)
            drain_all()

    nc.compile()
    return nc


def get_nc(repeat=1):
    key = f"nc{repeat}"
    if key not in _CACHE:
        _CACHE[key] = _build_nc(repeat)
    return _CACHE[key]


def make_in_maps(query, key_, value, Wq, bq, Wk, bk, Wv, bv, Wo, bo):
    bf = ml_dtypes.bfloat16
    f32 = np.float32
    query = np.asarray(query, f32)
    key_ = np.asarray(key_, f32)
    value = np.asarray(value, f32)
    Wq, Wk, Wv, Wo = (np.asarray(w, f32) for w in (Wq, Wk, Wv, Wo))
    bq, bk, bv = (np.asarray(x, f32) for x in (bq, bk, bv))

    xqT = [np.ascontiguousarray(query[b].T).astype(bf) for b in range(B)]
    xkT = [np.ascontiguousarray(key_[b].T).astype(bf) for b in range(B)]
    xvT = [np.ascontiguousarray(value[b].T).astype(bf) for b in range(B)]

    per_g = []
    for g in range(2):
        rows = slice(g * FG, (g + 1) * FG)
        per_g.append({
            "wqT": np.ascontiguousarray(Wq[rows].T).astype(bf),
            "wkT": np.ascontiguousarray(Wk[rows].T).astype(bf),
            "wvT": np.ascontiguousarray(Wv[rows].T).astype(bf),
            "woT": np.ascontiguousarray(Wo.T[rows]).astype(bf),
            "bq2": np.ascontiguousarray(bq[rows].reshape(MT, P).T),
            "bk2": np.ascontiguousarray(bk[rows].reshape(MT, P).T),
            "bvr": np.ascontiguousarray(bv[rows].reshape(1, FG)).astype(bf),
        })

    in_maps = []
    for c in range(NCORES):
        b, g = c // 2, c % 2
        m = {"xqT": xqT[b], "xkT": xkT[b], "xvT": xvT[b]}
        m.update(per_g[g])
        in_maps.append(m)
    return in_maps


def kernel(query, key_, value, Wq, bq, Wk, bk, Wv, bv, Wo, bo):
    from concourse.bass_utils import run_bass_kernel_spmd

    nc = get_nc()
    in_maps = make_in_maps(query, key_, value, Wq, bq, Wk, bk, Wv, bv, Wo, bo)
    res = run_bass_kernel_spmd(nc, in_maps, core_ids=list(range(NCORES)))
    zs = [res.results[c]["z"].astype(np.float32) for c in range(NCORES)]
    bo = np.asarray(bo, np.float32)
    out = np.stack([zs[2 * b] + zs[2 * b + 1] + bo[None, :] for b in range(B)])
    return out.astype(np.float32)

